# revision 1
# baseline (speedup 1.0000x reference)
"""BloomAttention (B=1, S=2048, H=4096, NH=32) on 8 Trainium2 cores — v3.

Head-parallel TP (4 heads/core), all matmul operands bf16 (fp32 PSUM accum).
 - QKV: PSUM-resident accumulation over the full 4096 contraction per
   256-wide seq chunk (no SBUF/DVE accumulate passes).
 - scores: QK matmul + a 3-row aux matmul (ones3 x [alibi_hi;mid;lo] — exact
   3-way bf16 split of alibi) + a tri^T x identity matmul for the causal
   diagonal block. Everything lands in PSUM; no DVE score pass at all.
 - softmax: exp straight from PSUM on the scalar engine with a
   host-precomputed per-query upper bound as bias (replaces the row max;
   the shift cancels in normalization), accum_out gives row sums.
 - probs normalized by 1/sum (DVE), transposed to key-major via DMA X-bar
   transposes (USE_DMA_T) or PE transposes + copies (fallback).
 - dense: row-parallel partials written bf16; host sums cores+bias+residual.
"""
import math
import numpy as np
from contextlib import ExitStack

import ml_dtypes

import concourse.bacc as bacc
import concourse.bass as bass
import concourse.mybir as mybir
import concourse.tile as tile
from concourse.bass_utils import run_bass_kernel_spmd

B, S, H, NH = 1, 2048, 4096, 32
HD = H // NH            # 128
NCORES = 8
HPC = NH // NCORES      # 4 heads per core
DPC = HPC * HD          # 512
INV_NORM = 1.0 / math.sqrt(HD)
NEG = -1.0e30
CPAD = 15.0             # slack above max alibi in b_q
P = 128
QB = S // P             # 16 query blocks
NCH = 8                 # seq chunks in phase 1
SCW = S // NCH          # 256 seq chunk width
KT = H // P             # 32 contraction tiles
F32 = mybir.dt.float32
BF16 = mybir.dt.bfloat16
ADD = mybir.AluOpType.add

USE_DMA_T = False        # DMA X-bar transposes vs PE transposes

_CACHE = {}


def _build(key):
    kNq, kLoT = key
    nc = bacc.Bacc("TRN2", target_bir_lowering=False, debug=False,
                   num_devices=NCORES)

    hpk = nc.dram_tensor("hpk", [NCH, P, KT * SCW], BF16, kind="ExternalInput")
    wqk = nc.dram_tensor("wqk", [2 * HPC, P, KT * P], BF16, kind="ExternalInput")
    wv = nc.dram_tensor("wv", [P, KT * DPC], BF16, kind="ExternalInput")
    bqk_t = nc.dram_tensor("bqk", [P, 2 * HPC], F32, kind="ExternalInput")
    bv_t = nc.dram_tensor("bv", [1, DPC], F32, kind="ExternalInput")
    alsp_t = nc.dram_tensor("alsp", [HPC, 3, S], BF16, kind="ExternalInput")
    trit_t = nc.dram_tensor("trit", [QB, P, P], BF16, kind="ExternalInput")
    bexp_t = nc.dram_tensor("bexp", [P, HPC * QB], F32, kind="ExternalInput")
    ident_t = nc.dram_tensor("ident", [P, P], BF16, kind="ExternalInput")
    wdp_t = nc.dram_tensor("wdp", [P, HPC * H], BF16, kind="ExternalInput")
    out_t = nc.dram_tensor("out_part", [S, H], BF16, kind="ExternalOutput")

    Ident = mybir.ActivationFunctionType.Identity
    Exp = mybir.ActivationFunctionType.Exp

    with tile.TileContext(nc) as tc, ExitStack() as top:
        persist = top.enter_context(tc.tile_pool(name="persist", bufs=1))
        qk_sb = [persist.tile([P, S], BF16, tag=f"qk_{f}", name=f"qk_{f}")
                 for f in range(2 * HPC)]                  # Q heads 0-3, K heads 0-3
        v_sb = [persist.tile([P, DPC], BF16, tag=f"v_{st}", name=f"v_{st}")
                for st in range(S // P)]
        ident_sb = persist.tile([P, P], BF16, tag="ident")
        bqk_sb = persist.tile([P, 2 * HPC], F32, tag="bqk")
        bexp_sb = persist.tile([P, HPC * QB], F32, tag="bexp")
        bv_bc = persist.tile([P, DPC], F32, tag="bv_bc")
        ones_all = persist.tile([P, P], BF16, tag="ones3")
        alsp_a = persist.tile([P, S], BF16, tag="alsp_a")
        alsp_b = persist.tile([3, S], BF16, tag="alsp_b")
        ones3 = [ones_all[32 * h:32 * h + 3, :] for h in range(3)] + \
            [ones_all[0:3, :]]
        alsp_sb = [alsp_a[32 * h:32 * h + 3, :] for h in range(3)] + [alsp_b]

        trit_sb = [persist.tile([P, P], BF16, tag=f"trit_{qb}",
                                name=f"trit_{qb}") for qb in range(QB)]
        ctx_sb = [persist.tile([P, S], BF16, tag=f"ctx_{h}", name=f"ctx_{h}")
                  for h in range(HPC)]
        sm_pool = top.enter_context(tc.tile_pool(name="small", bufs=8))

        prow_t = {}   # (h, qb) -> prob row tile
        quads = {}    # (h, qg) -> dict of quad tiles [P, 4, 512]

        def chain(h, qb, pool, mid_cb=None):
            """scores (+alibi, -b_q, mask) in PSUM -> exp -> normalized
            bf16 prob row. mid_cb (if given) is invoked after the second
            chunk so PE has filler work while exp drains the PSUM tiles."""
            kN = kNq[qb]
            lo0 = (kLoT[h][qb] * P) // 512 * 512
            nt = (kN - lo0 + 511) // 512
            prow = pool.tile([P, 512 if qb < 4 else S], BF16, tag="prow",
                             name=f"prow_{h}_{qb}")
            prow_t[(h, qb)] = prow
            strip = sm_pool.tile([P, 4], F32, tag="strip")
            rinv = sm_pool.tile([P, 1], F32, tag="rinv")
            nb = bexp_sb[:, h * QB + qb: h * QB + qb + 1]
            qst = qk_sb[h][:, qb * P:(qb + 1) * P]
            for ti in range(nt):
                lo = lo0 + 512 * ti
                N = min(512, kN - lo)
                ps = ps_sc.tile([P, 512], F32, tag="ps_sc")
                sl = ps[:, :N]
                diag = (lo + N == kN)
                nc.tensor.matmul(
                    sl, qst, qk_sb[HPC + h][:, lo:lo + N],
                    start=True, stop=False)
                nc.tensor.matmul(
                    sl, ones3[h],
                    alsp_sb[h][:, lo:lo + N],
                    start=False, stop=not diag)
                if diag:
                    nc.tensor.matmul(
                        ps[:, N - P:N],
                        trit_sb[qb], ident_sb,
                        start=False, stop=True)
                nc.scalar.activation(
                    out=prow[:, lo:lo + N], in_=sl,
                    func=Exp, bias=nb, scale=1.0,
                    accum_out=strip[:, ti:ti + 1])
                if mid_cb is not None and ti == min(1, nt - 1):
                    mid_cb()
                    mid_cb = None
            if mid_cb is not None:
                mid_cb()
            if nt > 1:
                tot = sm_pool.tile([P, 1], F32, tag="tot")
                nc.vector.tensor_reduce(
                    out=tot, in_=strip[:, :nt], op=ADD,
                    axis=mybir.AxisListType.X)
            else:
                tot = strip[:, 0:1]
            nc.vector.reciprocal(out=rinv, in_=tot)
            nc.vector.tensor_scalar_mul(
                out=prow[:, lo0:kN], in0=prow[:, lo0:kN], scalar1=rinv)

        # ---------------- phase 1: QKV projection ----------------
        with ExitStack() as ph1:
            wq_pool = ph1.enter_context(tc.tile_pool(name="wq", bufs=1))
            hid_pool = ph1.enter_context(tc.tile_pool(name="hid", bufs=2))
            psqk = ph1.enter_context(
                tc.tile_pool(name="psqk", bufs=5, space="PSUM"))
            psv = ph1.enter_context(
                tc.tile_pool(name="psv", bufs=3, space="PSUM"))

            wqk_sb = [wq_pool.tile([P, KT * P], BF16, tag=f"wqk_{f}",
                                   name=f"wqk_{f}") for f in range(2 * HPC)]
            wv_sb = wq_pool.tile([P, KT * DPC], BF16, tag="wv")
            hps = [hid_pool.tile([P, KT * SCW], BF16, tag="hp", name=f"hp_{c}")
                   for c in range(NCH)]
            # chunk-0's operands stream in use-order: bias, hidden chunk,
            # Q/K weights per fc, then V weights in halves
            nc.sync.dma_start(out=bqk_sb, in_=bqk_t[:, :])
            # stream chunk 0 + first Q weights piecewise so fc0 starts ~5us in
            q8 = 8 * SCW
            w16 = 16 * P
            nc.sync.dma_start(out=hps[0][:, :q8], in_=hpk[0][:, :q8])
            nc.sync.dma_start(out=wqk_sb[0][:, :w16], in_=wqk[0][:, :w16])
            nc.sync.dma_start(out=hps[0][:, q8:2 * q8], in_=hpk[0][:, q8:2 * q8])
            nc.sync.dma_start(out=hps[0][:, 2 * q8:3 * q8],
                              in_=hpk[0][:, 2 * q8:3 * q8])
            nc.sync.dma_start(out=wqk_sb[0][:, w16:], in_=wqk[0][:, w16:])
            nc.sync.dma_start(out=hps[0][:, 3 * q8:], in_=hpk[0][:, 3 * q8:])
            nc.sync.dma_start(out=wqk_sb[1][:, :w16], in_=wqk[1][:, :w16])
            nc.sync.dma_start(out=wqk_sb[1][:, w16:], in_=wqk[1][:, w16:])
            for f in range(2, 2 * HPC):
                nc.sync.dma_start(out=wqk_sb[f], in_=wqk[f])
            half = KT * DPC // 2
            nc.sync.dma_start(out=wv_sb[:, :half], in_=wv[:, :half])
            nc.sync.dma_start(out=wv_sb[:, half:], in_=wv[:, half:])
            nc.vector.memset(ones_all, 1.0)
            nc.gpsimd.dma_start(
                out=bv_bc,
                in_=bass.AP(tensor=bv_t, offset=0, ap=[[0, P], [1, DPC]]))

            for c in range(NCH):
                hp = hps[c]
                if c + 1 < NCH:
                    nc.sync.dma_start(out=hps[c + 1], in_=hpk[c + 1])
                if c == 1:
                    # attention-phase constants; emitted here so they queue
                    # behind the first hidden chunks, not ahead of them
                    nc.sync.dma_start(out=ident_sb, in_=ident_t[:, :])
                    nc.sync.dma_start(out=bexp_sb, in_=bexp_t[:, :])
                    for qb in range(QB):
                        nc.sync.dma_start(out=trit_sb[qb], in_=trit_t[qb])
                    for h in range(HPC):
                        nc.sync.dma_start(out=alsp_sb[h], in_=alsp_t[h])
                for f in range(2 * HPC):
                    ps = psqk.tile([P, SCW], F32, tag="psqk")
                    for j in range(KT):
                        nc.tensor.matmul(
                            ps, wqk_sb[f][:, j * P:(j + 1) * P],
                            hp[:, j * SCW:(j + 1) * SCW],
                            start=(j == 0), stop=(j == KT - 1))
                    nc.scalar.activation(
                        out=qk_sb[f][:, c * SCW:(c + 1) * SCW], in_=ps,
                        func=Ident, bias=bqk_sb[:, f:f + 1], scale=1.0)
                for st2 in range(SCW // P):
                    st = c * (SCW // P) + st2
                    ps = psv.tile([P, DPC], F32, tag="psv")
                    for j in range(KT):
                        nc.tensor.matmul(
                            ps, hp[:, j * SCW + st2 * P: j * SCW + (st2 + 1) * P],
                            wv_sb[:, j * DPC:(j + 1) * DPC],
                            start=(j == 0), stop=(j == KT - 1))
                    nc.vector.tensor_add(out=v_sb[st], in0=ps, in1=bv_bc)

        # ---------------- phases 2+3: attention + dense ----------------
        with ExitStack() as ph2:
            pr_pool = ph2.enter_context(tc.tile_pool(name="prow", bufs=9))
            pq_pool = ph2.enter_context(tc.tile_pool(name="pquad", bufs=9))
            wd_pool = ph2.enter_context(tc.tile_pool(name="wd", bufs=1))
            st_pool = ph2.enter_context(tc.tile_pool(name="ostage", bufs=2))
            ps_sc = ph2.enter_context(
                tc.tile_pool(name="ps_sc", bufs=3, space="PSUM"))
            ps_cx = ph2.enter_context(
                tc.tile_pool(name="ps_cx", bufs=1, space="PSUM"))
            ps_d = ph2.enter_context(
                tc.tile_pool(name="ps_d", bufs=2, space="PSUM"))
            if not USE_DMA_T:
                ps_st = ph2.enter_context(
                    tc.tile_pool(name="ps_st", bufs=2, space="PSUM"))

            wdp_sb = wd_pool.tile([P, HPC * H], BF16, tag="wdp")
            nc.sync.dma_start(out=wdp_sb, in_=wdp_t[:, :])


            def transposes(h, qb):
                """prow(h, qb) -> key-major quad slices."""
                kN = kNq[qb]
                qg, qbl = qb // 4, qb % 4
                if qbl == 0:
                    ntile_g = kNq[4 * qg + 3] // P
                    a0 = kLoT[h][4 * qg] // 4
                    quads[(h, qg)] = {
                        a: pq_pool.tile([P, 4, 512], BF16, tag="pquad",
                                        name=f"pq_{h}_{qg}_{a}")
                        for a in range(a0, (ntile_g + 3) // 4)}
                prow = prow_t.pop((h, qb))
                qlist = quads[(h, qg)]
                ntile = kN // P
                t0 = kLoT[h][qb]
                if USE_DMA_T:
                    for t in range(t0, ntile):
                        nc.sync.dma_start(
                            out=qlist[t // 4][:, t % 4, qbl * P:(qbl + 1) * P],
                            in_=prow[:, t * P:(t + 1) * P], transpose=True)
                else:
                    t = t0
                    while t < ntile:
                        t = (t // 4) * 4          # align to quad boundary
                        lo_t = max(t, t0)
                        cnt = min(8, ntile - t)
                        stg = ps_st.tile([P, 8, P], BF16, tag="stg")
                        for i in range(lo_t - t, cnt):
                            nc.tensor.transpose(
                                stg[:, i, :], prow[:, (t + i) * P:(t + i + 1) * P],
                                ident_sb)
                        for half in range((cnt + 3) // 4):
                            i0 = max(4 * half, lo_t - t)
                            i1 = min(4 * half + 4, cnt)
                            if i0 >= i1:
                                continue
                            nc.vector.tensor_copy(
                                out=qlist[t // 4 + half][:, i0 - 4 * half:
                                                         i1 - 4 * half,
                                                         qbl * P:(qbl + 1) * P],
                                in_=stg[:, i0:i1, :])
                        t += cnt

            def pv(h, qg):
                kns = [kNq[4 * qg + i] for i in range(4)]
                t0s = [kLoT[h][4 * qg + i] for i in range(4)]
                ntile = kns[3] // P
                tiles_per = [k // P for k in kns]
                qlist = quads.pop((h, qg))
                cps = ps_cx.tile([P, 512], F32, tag="ps_cx")
                nc.vector.memset(cps, 0.0)
                for t in range(t0s[0], ntile):
                    cov = [i for i in range(4)
                           if tiles_per[i] > t and t0s[i] <= t]
                    if not cov:
                        continue
                    ilo, ihi = cov[0], cov[-1]
                    nc.tensor.matmul(
                        cps[:, ilo * P:(ihi + 1) * P],
                        v_sb[t][:, h * P:(h + 1) * P],
                        qlist[t // 4][:, t % 4, ilo * P:(ihi + 1) * P],
                        start=False, stop=(t == ntile - 1),
                        skip_group_check=True)
                nc.vector.tensor_copy(
                    out=ctx_sb[h][:, qg * 512:(qg + 1) * 512], in_=cps)

            def dense_qb(qb, split_dma=False):
                    stage = st_pool.tile([P, H], BF16, tag="ostage")
                    if split_dma:
                        pass
                    for oc in range(8):
                        ps = ps_d.tile([P, 512], F32, tag="ps_d")
                        for hh in range(HPC):
                            nc.tensor.matmul(
                                ps, ctx_sb[hh][:, qb * P:(qb + 1) * P],
                                wdp_sb[:, hh * H + oc * 512:
                                       hh * H + (oc + 1) * 512],
                                start=(hh == 0), stop=(hh == HPC - 1))
                        if oc % 2 == 0:
                            nc.vector.tensor_copy(
                                out=stage[:, oc * 512:(oc + 1) * 512], in_=ps)
                        else:
                            nc.scalar.copy(
                                out=stage[:, oc * 512:(oc + 1) * 512], in_=ps)
                        if split_dma and oc == 3:
                            nc.sync.dma_start(
                                out=out_t[qb * P:(qb + 1) * P, :H // 2],
                                in_=stage[:, :H // 2])
                    if split_dma:
                        nc.sync.dma_start(
                            out=out_t[qb * P:(qb + 1) * P, H // 2:],
                            in_=stage[:, H // 2:])
                    else:
                        nc.sync.dma_start(
                            out=out_t[qb * P:(qb + 1) * P, :], in_=stage)

            # software-pipelined emission over 16 (qg, h) units, head-major
            # within each query group. Unit-level lag: unit u's four chains
            # are interleaved with unit u-1's transposes; u-1's PV closes at
            # qbl 3. Dense query blocks are spread one-or-two per unit as
            # their query group's ctx completes.
            units = [(qg, h) for qg in range(4) for h in range(HPC)]
            dq = []        # dense qbs ready to emit
            for u, (qg, h) in enumerate(units):
                prev = units[u - 1] if u > 0 else None
                for qbl in range(4):
                    steps = []
                    if prev:
                        steps.append(lambda ph=prev[1], pq=4 * prev[0] + qbl:
                                     transposes(ph, pq))
                    if qbl == 2 and dq:
                        steps.append(lambda q=dq.pop(0): dense_qb(q))
                    cb = (lambda ss=tuple(steps): [s() for s in ss]) \
                        if steps else None
                    chain(h, 4 * qg + qbl, pr_pool, mid_cb=cb)
                    if qbl == 3 and prev:
                        pv(prev[1], prev[0])
                        if prev[1] == HPC - 1:
                            dq.extend(4 * prev[0] + i for i in range(4))
                if len(dq) > 4:
                    dense_qb(dq.pop(0))
            prev = units[-1]
            for qbl in range(4):
                transposes(prev[1], 4 * prev[0] + qbl)
            pv(prev[1], prev[0])
            dq.extend(4 * prev[0] + i for i in range(4))
            for i, qb in enumerate(dq):
                dense_qb(qb, split_dma=(i == len(dq) - 1))

    nc.compile()
    return nc


def _host_prep(hidden_states, alibi, attention_mask, w_qkv, b_qkv, w_dense):
    """Returns (kNq, in_maps) for the 8 cores."""
    hidden = np.asarray(hidden_states, np.float32).reshape(S, H)
    mask = np.asarray(attention_mask).reshape(S, S)
    alibi = np.asarray(alibi, np.float32).reshape(NH, S)
    w_qkv = np.asarray(w_qkv, np.float32)
    b_qkv = np.asarray(b_qkv, np.float32)
    w_dense = np.asarray(w_dense, np.float32)

    allowed = ~mask
    assert allowed.any(axis=1).all(), "fully-masked row"
    limit = S - np.argmax(allowed[:, ::-1], axis=1)      # last allowed + 1
    recon = np.arange(S)[None, :] >= limit[:, None]
    if not np.array_equal(mask, recon):
        raise NotImplementedError("mask is not suffix-structured")
    kNq = []
    for qb in range(QB):
        lb = limit[qb * P:(qb + 1) * P]
        kN = int(math.ceil(lb.max() / P) * P)
        if lb.min() < kN - P:
            raise NotImplementedError("mask boundary spans >128 cols in block")
        kNq.append(kN)
    if any(kNq[i] > kNq[i + 1] for i in range(QB - 1)):
        raise NotImplementedError("non-monotone key ranges")

    bf = ml_dtypes.bfloat16
    hpk = np.ascontiguousarray(
        hidden.reshape(NCH, SCW, KT, P).transpose(0, 3, 2, 1)
    ).reshape(NCH, P, KT * SCW).astype(bf)
    ident = np.eye(P, dtype=np.float32).astype(bf)
    col = np.arange(S)

    # causal diagonal mask tiles, transposed for use as matmul stationary:
    # trit[qb][k, q] = NEG where key kN-P+k is masked for query q
    trit = np.zeros((QB, P, P), np.float32)
    for qb in range(QB):
        kN = kNq[qb]
        lb = limit[qb * P:(qb + 1) * P]
        cc = col[kN - P:kN]
        trit[qb] = np.where(cc[:, None] >= lb[None, :], NEG, 0.0)
    trit = trit.astype(bf)

    wr = w_qkv.reshape(NH, 3, HD, H)
    br = b_qkv.reshape(NH, 3, HD)

    in_maps = []
    all_kLoT = None
    for c in range(NCORES):
        heads = [c + NCORES * j for j in range(HPC)]
        hs = np.asarray(heads)
        Wq = wr[hs, 0].reshape(DPC, H) * INV_NORM
        Wk = wr[hs, 1].reshape(DPC, H)
        Wv = wr[hs, 2].reshape(DPC, H)
        WQK = np.concatenate([Wq, Wk], axis=0)           # [1024, H]
        wqk_c = np.ascontiguousarray(
            WQK.reshape(2 * HPC, P, KT, P).transpose(0, 3, 2, 1)
        ).reshape(2 * HPC, P, KT * P).astype(bf)
        wv_c = np.ascontiguousarray(
            Wv.reshape(DPC, KT, P).transpose(2, 1, 0)
        ).reshape(P, KT * DPC).astype(bf)
        bq = br[hs, 0].reshape(-1) * INV_NORM
        bk = br[hs, 1].reshape(-1)
        bqk_c = np.ascontiguousarray(
            np.concatenate([bq, bk]).reshape(2 * HPC, P).T)
        bv_c = br[hs, 2].reshape(1, DPC)

        al_c = alibi[hs].astype(np.float32)               # [HPC, S]
        # exact 3-way bf16 split of alibi
        a_hi = al_c.astype(bf).astype(np.float32)
        r1 = al_c - a_hi
        a_mid = r1.astype(bf).astype(np.float32)
        a_lo = (r1 - a_mid).astype(bf)
        alsp_c = np.stack(
            [a_hi.astype(bf), a_mid.astype(bf), a_lo], axis=1)  # [HPC,3,S]

        cmax = np.maximum.accumulate(al_c, axis=1)
        bexp_c = np.zeros((P, HPC * QB), np.float32)
        kLoT_c = []
        for h in range(HPC):
            b_row = cmax[h, limit - 1] + CPAD
            klo_h = []
            for qb in range(QB):
                bexp_c[:, h * QB + qb] = -b_row[qb * P:(qb + 1) * P]
                # keys whose softmax weight is < ~e^-19 for every query in
                # the block (qk slack 25 + prob floor e^-23): contribute
                # < 1e-5 total mass, far below the kernel's 5e-3 error
                bmin = b_row[qb * P:(qb + 1) * P].min()
                live = al_c[h] >= (bmin - 40.0)
                k0 = int(np.argmax(live)) if live.any() else 0
                klo_h.append(min(k0 // P, kNq[qb] // P - 1))
            kLoT_c.append(tuple(klo_h))
        kLoT_c = tuple(kLoT_c)
        if all_kLoT is None:
            all_kLoT = kLoT_c
        else:
            # one SPMD program for all cores: take the elementwise min
            all_kLoT = tuple(
                tuple(min(a, b) for a, b in zip(ra, rb))
                for ra, rb in zip(all_kLoT, kLoT_c))
        dcols = np.concatenate(
            [np.arange(g * HD, (g + 1) * HD) for g in heads])
        wdp_c = np.ascontiguousarray(
            w_dense[:, dcols].reshape(H, HPC, P)
            .transpose(2, 1, 0)).reshape(P, HPC * H).astype(bf)
        in_maps.append({
            "hpk": hpk, "wqk": wqk_c, "wv": wv_c, "bqk": bqk_c, "bv": bv_c,
            "alsp": alsp_c, "trit": trit, "bexp": bexp_c, "ident": ident,
            "wdp": wdp_c,
        })
    return (tuple(kNq), all_kLoT), in_maps


def kernel(hidden_states, residual, alibi, attention_mask,
           w_qkv, b_qkv, w_dense, b_dense):
    key, in_maps = _host_prep(hidden_states, alibi, attention_mask,
                              w_qkv, b_qkv, w_dense)
    if key not in _CACHE:
        _CACHE[key] = _build(key)
    nc = _CACHE[key]
    res = run_bass_kernel_spmd(nc, in_maps, list(range(NCORES)))
    acc = res.results[0]["out_part"].astype(np.float32)
    for c in range(1, NCORES):
        acc += res.results[c]["out_part"].astype(np.float32)
    out = acc + np.asarray(b_dense, np.float32)[None, :]
    out = out + np.asarray(residual, np.float32).reshape(S, H)
    return out.reshape(B, S, H).astype(np.float32)



# revision 4
# speedup vs baseline: 1.2026x; 1.2026x over previous
"""BloomAttention (B=1, S=2048, H=4096, NH=32) on 8 Trainium2 cores — v4.

Head-parallel TP (4 heads/core). v4 converts the two big dense GEMMs (QKV
projection and output dense) to fp8-e4m3 DoubleRow matmuls with an exact
hi+lo error split (3-term: hi*hi + hi*lo + lo*hi), halving PE time per
term pair vs bf16 for a net 0.75x on those stages:
 - QKV: hidden (x8) and weights (x512) split to fp8 hi/lo on the host;
   48 DoubleRow matmuls (16 k-pairs x 3 terms) accumulate the full 4096
   contraction in PSUM; ACT descales by 1/4096 while adding the bias.
 - V: same; descale+bias in one DVE scalar_tensor_tensor op.
 - scores: bf16 QK matmul + 3-row aux matmul (ones3 x alibi split) +
   tri^T x identity for the causal diagonal block, all in PSUM.
 - softmax: exp from PSUM on ACT with host-precomputed per-query upper
   bound as bias; accum_out row sums; DVE normalize; PE transposes.
 - ctx: PV in bf16; PSUM result split on-device to fp8 hi/lo (copy +
   subtract) for the dense stage.
 - dense: fp8x3 DoubleRow over hh-pairs, row-parallel partials written
   bf16 (1/512 descale on the PSUM->stage copies); host sums cores+bias+
   residual.
"""
import math
import numpy as np
from contextlib import ExitStack

import ml_dtypes

import concourse.bacc as bacc
import concourse.bass as bass
import concourse.mybir as mybir
import concourse.tile as tile
from concourse.bass_utils import run_bass_kernel_spmd

B, S, H, NH = 1, 2048, 4096, 32
HD = H // NH            # 128
NCORES = 8
HPC = NH // NCORES      # 4 heads per core
DPC = HPC * HD          # 512
INV_NORM = 1.0 / math.sqrt(HD)
NEG = -1.0e30
CPAD = 15.0             # slack above max alibi in b_q
P = 128
QB = S // P             # 16 query blocks
NCH = 8                 # seq chunks in phase 1
SCW = S // NCH          # 256 seq chunk width
KT = H // P             # 32 contraction tiles
KP = KT // 2            # 16 DoubleRow k-pairs
SX = 8.0                # fp8 scale for hidden
SW = 512.0              # fp8 scale for weights
DSC = 1.0 / (SX * SW)   # PSUM descale
F32 = mybir.dt.float32
BF16 = mybir.dt.bfloat16
FP8 = mybir.dt.float8e4
ADD = mybir.AluOpType.add
MUL = mybir.AluOpType.mult
DR = mybir.MatmulPerfMode.DoubleRow

USE_DMA_T = False        # DMA X-bar transposes vs PE transposes

_CACHE = {}


def _build(key):
    kNq, kLoT = key
    nc = bacc.Bacc("TRN2", target_bir_lowering=False, debug=False,
                   num_devices=NCORES)

    hpk_hi = nc.dram_tensor("hpk_hi", [NCH, P, KT * SCW], FP8,
                            kind="ExternalInput")
    hpk_lo = nc.dram_tensor("hpk_lo", [NCH, P, KT * SCW], FP8,
                            kind="ExternalInput")
    wqk_hi = nc.dram_tensor("wqk_hi", [2 * HPC, P, KT * P], FP8,
                            kind="ExternalInput")
    wqk_lo = nc.dram_tensor("wqk_lo", [2 * HPC, P, KT * P], FP8,
                            kind="ExternalInput")
    wv_hi = nc.dram_tensor("wv_hi", [P, KT * DPC], FP8, kind="ExternalInput")
    wv_lo = nc.dram_tensor("wv_lo", [P, KT * DPC], FP8, kind="ExternalInput")
    bqk_t = nc.dram_tensor("bqk", [P, 2 * HPC], F32, kind="ExternalInput")
    bv_t = nc.dram_tensor("bv", [1, DPC], F32, kind="ExternalInput")
    alsp_t = nc.dram_tensor("alsp", [HPC, 3, S], BF16, kind="ExternalInput")
    trit_t = nc.dram_tensor("trit", [QB, P, P], BF16, kind="ExternalInput")
    bexp_t = nc.dram_tensor("bexp", [P, HPC * QB], F32, kind="ExternalInput")
    ident_t = nc.dram_tensor("ident", [P, P], BF16, kind="ExternalInput")
    wdp_hi = nc.dram_tensor("wdp_hi", [P, HPC * H], FP8, kind="ExternalInput")
    wdp_lo = nc.dram_tensor("wdp_lo", [P, HPC * H], FP8, kind="ExternalInput")
    out_t = nc.dram_tensor("out_part", [S, H], BF16, kind="ExternalOutput")

    Ident = mybir.ActivationFunctionType.Identity
    Exp = mybir.ActivationFunctionType.Exp

    with tile.TileContext(nc) as tc, ExitStack() as top:
        persist = top.enter_context(tc.tile_pool(name="persist", bufs=1))
        qk_sb = [persist.tile([P, S], BF16, tag=f"qk_{f}", name=f"qk_{f}")
                 for f in range(2 * HPC)]                  # Q heads 0-3, K heads 0-3
        v_sb = [persist.tile([P, DPC], BF16, tag=f"v_{st}", name=f"v_{st}")
                for st in range(S // P)]
        ident_sb = persist.tile([P, P], BF16, tag="ident")
        bqk_sb = persist.tile([P, 2 * HPC], F32, tag="bqk")
        bexp_sb = persist.tile([P, HPC * QB], F32, tag="bexp")
        bv_bc = persist.tile([P, DPC], F32, tag="bv_bc")
        ones_all = persist.tile([P, P], BF16, tag="ones3")
        alsp_a = persist.tile([P, S], BF16, tag="alsp_a")
        alsp_b = persist.tile([3, S], BF16, tag="alsp_b")
        ones3 = [ones_all[32 * h:32 * h + 3, :] for h in range(3)] + \
            [ones_all[0:3, :]]
        alsp_sb = [alsp_a[32 * h:32 * h + 3, :] for h in range(3)] + [alsp_b]

        trit_sb = [persist.tile([P, P], BF16, tag=f"trit_{qb}",
                                name=f"trit_{qb}") for qb in range(QB)]
        # ctx in fp8 hi/lo, hh-major so hh-pairs are stride-S slices
        ctx_hi = persist.tile([P, HPC, S], FP8, tag="ctx_hi")
        ctx_lo = persist.tile([P, HPC, S], FP8, tag="ctx_lo")
        sm_pool = top.enter_context(tc.tile_pool(name="small", bufs=8))

        prow_t = {}   # (h, qb) -> prob row tile
        quads = {}    # (h, qg) -> dict of quad tiles [P, 4, 512]

        def chain(h, qb, pool, mid_cb=None):
            """scores (+alibi, -b_q, mask) in PSUM -> exp -> normalized
            bf16 prob row. mid_cb (if given) is invoked after the second
            chunk so PE has filler work while exp drains the PSUM tiles."""
            kN = kNq[qb]
            lo0 = (kLoT[h][qb] * P) // 512 * 512
            nt = (kN - lo0 + 511) // 512
            prow = pool.tile([P, 512 if qb < 4 else S], BF16, tag="prow",
                             name=f"prow_{h}_{qb}")
            prow_t[(h, qb)] = prow
            strip = sm_pool.tile([P, 4], F32, tag="strip")
            rinv = sm_pool.tile([P, 1], F32, tag="rinv")
            nb = bexp_sb[:, h * QB + qb: h * QB + qb + 1]
            qst = qk_sb[h][:, qb * P:(qb + 1) * P]
            for ti in range(nt):
                lo = lo0 + 512 * ti
                N = min(512, kN - lo)
                ps = ps_sc.tile([P, 512], F32, tag="ps_sc")
                sl = ps[:, :N]
                diag = (lo + N == kN)
                nc.tensor.matmul(
                    sl, qst, qk_sb[HPC + h][:, lo:lo + N],
                    start=True, stop=False)
                nc.tensor.matmul(
                    sl, ones3[h],
                    alsp_sb[h][:, lo:lo + N],
                    start=False, stop=not diag)
                if diag:
                    nc.tensor.matmul(
                        ps[:, N - P:N],
                        trit_sb[qb], ident_sb,
                        start=False, stop=True)
                nc.scalar.activation(
                    out=prow[:, lo:lo + N], in_=sl,
                    func=Exp, bias=nb, scale=1.0,
                    accum_out=strip[:, ti:ti + 1])
                if mid_cb is not None and ti == min(1, nt - 1):
                    mid_cb()
                    mid_cb = None
            if mid_cb is not None:
                mid_cb()
            if nt > 1:
                tot = sm_pool.tile([P, 1], F32, tag="tot")
                nc.vector.tensor_reduce(
                    out=tot, in_=strip[:, :nt], op=ADD,
                    axis=mybir.AxisListType.X)
            else:
                tot = strip[:, 0:1]
            nc.vector.reciprocal(out=rinv, in_=tot)
            nc.vector.tensor_scalar_mul(
                out=prow[:, lo0:kN], in0=prow[:, lo0:kN], scalar1=rinv)

        # ---------------- phase 1: QKV projection ----------------
        with ExitStack() as ph1:
            wq_pool = ph1.enter_context(tc.tile_pool(name="wq", bufs=1))
            hid_pool = ph1.enter_context(tc.tile_pool(name="hid", bufs=2))
            psqk = ph1.enter_context(
                tc.tile_pool(name="psqk", bufs=5, space="PSUM"))
            psv = ph1.enter_context(
                tc.tile_pool(name="psv", bufs=3, space="PSUM"))

            wqkh_sb = [wq_pool.tile([P, KT, P], FP8, tag=f"wqkh_{f}",
                                    name=f"wqkh_{f}") for f in range(2 * HPC)]
            wqkl_sb = [wq_pool.tile([P, KT, P], FP8, tag=f"wqkl_{f}",
                                    name=f"wqkl_{f}") for f in range(2 * HPC)]
            wvh_sb = wq_pool.tile([P, KT, DPC], FP8, tag="wvh")
            wvl_sb = wq_pool.tile([P, KT, DPC], FP8, tag="wvl")
            hph = [hid_pool.tile([P, KT, SCW], FP8, tag="hph",
                                 name=f"hph_{c}") for c in range(NCH)]
            hpl = [hid_pool.tile([P, KT, SCW], FP8, tag="hpl",
                                 name=f"hpl_{c}") for c in range(NCH)]
            # chunk-0's operands stream in use-order: bias, hidden-hi chunk
            # + Q/K hi weights (hi*hi terms start), then lo tensors
            nc.sync.dma_start(out=bqk_sb, in_=bqk_t[:, :])
            q8 = 8 * SCW
            w16 = 16 * P
            nc.sync.dma_start(out=hph[0][:, :8, :], in_=hpk_hi[0][:, :q8])
            nc.sync.dma_start(out=wqkh_sb[0][:, :16, :],
                              in_=wqk_hi[0][:, :w16])
            nc.sync.dma_start(out=hph[0][:, 8:16, :],
                              in_=hpk_hi[0][:, q8:2 * q8])
            nc.sync.dma_start(out=wqkh_sb[0][:, 16:, :],
                              in_=wqk_hi[0][:, w16:])
            nc.sync.dma_start(out=hph[0][:, 16:, :], in_=hpk_hi[0][:, 2 * q8:])
            nc.sync.dma_start(out=wqkl_sb[0], in_=wqk_lo[0])
            nc.sync.dma_start(out=hpl[0][:, :16, :], in_=hpk_lo[0][:, :2 * q8])
            nc.sync.dma_start(out=hpl[0][:, 16:, :], in_=hpk_lo[0][:, 2 * q8:])
            nc.sync.dma_start(out=wqkh_sb[1], in_=wqk_hi[1])
            nc.sync.dma_start(out=wqkl_sb[1], in_=wqk_lo[1])
            for f in range(2, 2 * HPC):
                nc.sync.dma_start(out=wqkh_sb[f], in_=wqk_hi[f])
                nc.sync.dma_start(out=wqkl_sb[f], in_=wqk_lo[f])
            half = KT * DPC // 2
            nc.sync.dma_start(out=wvh_sb[:, :16, :], in_=wv_hi[:, :half])
            nc.sync.dma_start(out=wvh_sb[:, 16:, :], in_=wv_hi[:, half:])
            nc.sync.dma_start(out=wvl_sb[:, :16, :], in_=wv_lo[:, :half])
            nc.sync.dma_start(out=wvl_sb[:, 16:, :], in_=wv_lo[:, half:])
            nc.vector.memset(ones_all, 1.0)
            nc.gpsimd.dma_start(
                out=bv_bc,
                in_=bass.AP(tensor=bv_t, offset=0, ap=[[0, P], [1, DPC]]))

            for c in range(NCH):
                xh, xl = hph[c], hpl[c]
                if c + 1 < NCH:
                    nc.sync.dma_start(out=hph[c + 1], in_=hpk_hi[c + 1])
                    nc.sync.dma_start(out=hpl[c + 1], in_=hpk_lo[c + 1])
                if c == 1:
                    # attention-phase constants; emitted here so they queue
                    # behind the first hidden chunks, not ahead of them
                    nc.sync.dma_start(out=ident_sb, in_=ident_t[:, :])
                    nc.sync.dma_start(out=bexp_sb, in_=bexp_t[:, :])
                    for qb in range(QB):
                        nc.sync.dma_start(out=trit_sb[qb], in_=trit_t[qb])
                    for h in range(HPC):
                        nc.sync.dma_start(out=alsp_sb[h], in_=alsp_t[h])
                for f in range(2 * HPC):
                    ps = psqk.tile([P, SCW], F32, tag="psqk")
                    terms = [(wqkh_sb[f], xh), (wqkl_sb[f], xh),
                             (wqkh_sb[f], xl)]
                    n3 = 3 * KP
                    for i3, (w3, x3) in enumerate(
                            (w, x) for (w, x) in terms for _ in range(KP)):
                        jp = i3 % KP
                        nc.tensor.matmul(
                            ps, w3[:, 2 * jp:2 * jp + 2, :],
                            x3[:, 2 * jp:2 * jp + 2, :],
                            start=(i3 == 0), stop=(i3 == n3 - 1),
                            perf_mode=DR)
                    nc.scalar.activation(
                        out=qk_sb[f][:, c * SCW:(c + 1) * SCW], in_=ps,
                        func=Ident, bias=bqk_sb[:, f:f + 1], scale=DSC)
                for st2 in range(SCW // P):
                    st = c * (SCW // P) + st2
                    ps = psv.tile([P, DPC], F32, tag="psv")
                    for vh in range(2):
                        psl = ps[:, vh * 256:(vh + 1) * 256]
                        terms = [(xh, wvh_sb), (xh, wvl_sb), (xl, wvh_sb)]
                        for i3, (x3, w3) in enumerate(
                                (x, w) for (x, w) in terms
                                for _ in range(KP)):
                            jp = i3 % KP
                            nc.tensor.matmul(
                                psl,
                                x3[:, 2 * jp:2 * jp + 2,
                                   st2 * P:(st2 + 1) * P],
                                w3[:, 2 * jp:2 * jp + 2,
                                   vh * 256:(vh + 1) * 256],
                                start=(i3 == 0), stop=(i3 == 3 * KP - 1),
                                perf_mode=DR)
                    nc.vector.scalar_tensor_tensor(
                        out=v_sb[st], in0=ps, scalar=DSC, in1=bv_bc,
                        op0=MUL, op1=ADD)

        # ---------------- phases 2+3: attention + dense ----------------
        with ExitStack() as ph2:
            pr_pool = ph2.enter_context(tc.tile_pool(name="prow", bufs=9))
            pq_pool = ph2.enter_context(tc.tile_pool(name="pquad", bufs=9))
            wd_pool = ph2.enter_context(tc.tile_pool(name="wd", bufs=1))
            st_pool = ph2.enter_context(tc.tile_pool(name="ostage", bufs=2))
            ps_sc = ph2.enter_context(
                tc.tile_pool(name="ps_sc", bufs=3, space="PSUM"))
            ps_cx = ph2.enter_context(
                tc.tile_pool(name="ps_cx", bufs=1, space="PSUM"))
            ps_d = ph2.enter_context(
                tc.tile_pool(name="ps_d", bufs=2, space="PSUM"))
            if not USE_DMA_T:
                ps_st = ph2.enter_context(
                    tc.tile_pool(name="ps_st", bufs=2, space="PSUM"))

            wdh_sb = wd_pool.tile([P, HPC, H], FP8, tag="wdh")
            wdl_sb = wd_pool.tile([P, HPC, H], FP8, tag="wdl")
            nc.sync.dma_start(out=wdh_sb, in_=wdp_hi[:, :])
            nc.sync.dma_start(out=wdl_sb, in_=wdp_lo[:, :])


            def transposes(h, qb):
                """prow(h, qb) -> key-major quad slices."""
                kN = kNq[qb]
                qg, qbl = qb // 4, qb % 4
                if qbl == 0:
                    ntile_g = kNq[4 * qg + 3] // P
                    a0 = kLoT[h][4 * qg] // 4
                    quads[(h, qg)] = {
                        a: pq_pool.tile([P, 4, 512], BF16, tag="pquad",
                                        name=f"pq_{h}_{qg}_{a}")
                        for a in range(a0, (ntile_g + 3) // 4)}
                prow = prow_t.pop((h, qb))
                qlist = quads[(h, qg)]
                ntile = kN // P
                t0 = kLoT[h][qb]
                if USE_DMA_T:
                    for t in range(t0, ntile):
                        nc.sync.dma_start(
                            out=qlist[t // 4][:, t % 4, qbl * P:(qbl + 1) * P],
                            in_=prow[:, t * P:(t + 1) * P], transpose=True)
                else:
                    t = t0
                    while t < ntile:
                        t = (t // 4) * 4          # align to quad boundary
                        lo_t = max(t, t0)
                        cnt = min(8, ntile - t)
                        stg = ps_st.tile([P, 8, P], BF16, tag="stg")
                        for i in range(lo_t - t, cnt):
                            nc.tensor.transpose(
                                stg[:, i, :], prow[:, (t + i) * P:(t + i + 1) * P],
                                ident_sb)
                        for half in range((cnt + 3) // 4):
                            i0 = max(4 * half, lo_t - t)
                            i1 = min(4 * half + 4, cnt)
                            if i0 >= i1:
                                continue
                            nc.vector.tensor_copy(
                                out=qlist[t // 4 + half][:, i0 - 4 * half:
                                                         i1 - 4 * half,
                                                         qbl * P:(qbl + 1) * P],
                                in_=stg[:, i0:i1, :])
                        t += cnt

            def pv(h, qg):
                kns = [kNq[4 * qg + i] for i in range(4)]
                t0s = [kLoT[h][4 * qg + i] for i in range(4)]
                ntile = kns[3] // P
                tiles_per = [k // P for k in kns]
                qlist = quads.pop((h, qg))
                cps = ps_cx.tile([P, 512], F32, tag="ps_cx")
                nc.vector.memset(cps, 0.0)
                for t in range(t0s[0], ntile):
                    cov = [i for i in range(4)
                           if tiles_per[i] > t and t0s[i] <= t]
                    if not cov:
                        continue
                    ilo, ihi = cov[0], cov[-1]
                    nc.tensor.matmul(
                        cps[:, ilo * P:(ihi + 1) * P],
                        v_sb[t][:, h * P:(h + 1) * P],
                        qlist[t // 4][:, t % 4, ilo * P:(ihi + 1) * P],
                        start=False, stop=(t == ntile - 1),
                        skip_group_check=True)
                nc.vector.tensor_copy(
                    out=ctx_hi[:, h, qg * 512:(qg + 1) * 512], in_=cps)
                nc.vector.tensor_sub(
                    out=ctx_lo[:, h, qg * 512:(qg + 1) * 512], in0=cps,
                    in1=ctx_hi[:, h, qg * 512:(qg + 1) * 512])

            def dense_qb(qb, split_dma=False):
                    stage = st_pool.tile([P, H], BF16, tag="ostage")
                    for oc in range(8):
                        ps = ps_d.tile([P, 512], F32, tag="ps_d")
                        for dh in range(2):
                            psl = ps[:, dh * 256:(dh + 1) * 256]
                            o0 = oc * 512 + dh * 256
                            i3 = 0
                            for hp2 in (0, 2):
                                for (ca, wb) in ((ctx_hi, wdh_sb),
                                                 (ctx_hi, wdl_sb),
                                                 (ctx_lo, wdh_sb)):
                                    nc.tensor.matmul(
                                        psl,
                                        ca[:, hp2:hp2 + 2,
                                           qb * P:(qb + 1) * P],
                                        wb[:, hp2:hp2 + 2, o0:o0 + 256],
                                        start=(i3 == 0), stop=(i3 == 5),
                                        perf_mode=DR)
                                    i3 += 1
                        if oc % 2 == 0:
                            nc.vector.tensor_scalar_mul(
                                out=stage[:, oc * 512:(oc + 1) * 512],
                                in0=ps, scalar1=1.0 / SW)
                        else:
                            nc.scalar.mul(
                                out=stage[:, oc * 512:(oc + 1) * 512],
                                in_=ps, mul=1.0 / SW)
                        if split_dma and oc == 3:
                            nc.sync.dma_start(
                                out=out_t[qb * P:(qb + 1) * P, :H // 2],
                                in_=stage[:, :H // 2])
                    if split_dma:
                        nc.sync.dma_start(
                            out=out_t[qb * P:(qb + 1) * P, H // 2:],
                            in_=stage[:, H // 2:])
                    else:
                        nc.sync.dma_start(
                            out=out_t[qb * P:(qb + 1) * P, :], in_=stage)

            # software-pipelined emission over 16 (qg, h) units, head-major
            # within each query group. Unit-level lag: unit u's four chains
            # are interleaved with unit u-1's transposes; u-1's PV closes at
            # qbl 3. Dense query blocks are spread one-or-two per unit as
            # their query group's ctx completes.
            units = [(qg, h) for qg in range(4) for h in range(HPC)]
            dq = []        # dense qbs ready to emit
            for u, (qg, h) in enumerate(units):
                prev = units[u - 1] if u > 0 else None
                for qbl in range(4):
                    steps = []
                    if prev:
                        steps.append(lambda ph=prev[1], pq=4 * prev[0] + qbl:
                                     transposes(ph, pq))
                    if qbl == 2 and dq:
                        steps.append(lambda q=dq.pop(0): dense_qb(q))
                    cb = (lambda ss=tuple(steps): [s() for s in ss]) \
                        if steps else None
                    chain(h, 4 * qg + qbl, pr_pool, mid_cb=cb)
                    if qbl == 3 and prev:
                        pv(prev[1], prev[0])
                        if prev[1] == HPC - 1:
                            dq.extend(4 * prev[0] + i for i in range(4))
                if len(dq) > 4:
                    dense_qb(dq.pop(0))
            prev = units[-1]
            for qbl in range(4):
                transposes(prev[1], 4 * prev[0] + qbl)
            pv(prev[1], prev[0])
            dq.extend(4 * prev[0] + i for i in range(4))
            for i, qb in enumerate(dq):
                dense_qb(qb, split_dma=(i == len(dq) - 1))

    nc.compile()
    return nc


def _split8(x, sc):
    """Exact hi+lo fp8-e4m3 split of x*sc. Returns (hi, lo) fp8 arrays."""
    f8 = ml_dtypes.float8_e4m3
    hi = (x * sc).astype(f8)
    lo = (x * sc - hi.astype(np.float32)).astype(f8)
    return hi, lo


def _host_prep(hidden_states, alibi, attention_mask, w_qkv, b_qkv, w_dense):
    """Returns (kNq, in_maps) for the 8 cores."""
    hidden = np.asarray(hidden_states, np.float32).reshape(S, H)
    mask = np.asarray(attention_mask).reshape(S, S)
    alibi = np.asarray(alibi, np.float32).reshape(NH, S)
    w_qkv = np.asarray(w_qkv, np.float32)
    b_qkv = np.asarray(b_qkv, np.float32)
    w_dense = np.asarray(w_dense, np.float32)

    allowed = ~mask
    assert allowed.any(axis=1).all(), "fully-masked row"
    limit = S - np.argmax(allowed[:, ::-1], axis=1)      # last allowed + 1
    recon = np.arange(S)[None, :] >= limit[:, None]
    if not np.array_equal(mask, recon):
        raise NotImplementedError("mask is not suffix-structured")
    kNq = []
    for qb in range(QB):
        lb = limit[qb * P:(qb + 1) * P]
        kN = int(math.ceil(lb.max() / P) * P)
        if lb.min() < kN - P:
            raise NotImplementedError("mask boundary spans >128 cols in block")
        kNq.append(kN)
    if any(kNq[i] > kNq[i + 1] for i in range(QB - 1)):
        raise NotImplementedError("non-monotone key ranges")

    bf = ml_dtypes.bfloat16
    hp_t = np.ascontiguousarray(
        hidden.reshape(NCH, SCW, KT, P).transpose(0, 3, 2, 1)
    ).reshape(NCH, P, KT * SCW)
    hpk_hi, hpk_lo = _split8(hp_t, SX)
    ident = np.eye(P, dtype=np.float32).astype(bf)
    col = np.arange(S)

    # causal diagonal mask tiles, transposed for use as matmul stationary:
    # trit[qb][k, q] = NEG where key kN-P+k is masked for query q
    trit = np.zeros((QB, P, P), np.float32)
    for qb in range(QB):
        kN = kNq[qb]
        lb = limit[qb * P:(qb + 1) * P]
        cc = col[kN - P:kN]
        trit[qb] = np.where(cc[:, None] >= lb[None, :], NEG, 0.0)
    trit = trit.astype(bf)

    wr = w_qkv.reshape(NH, 3, HD, H)
    br = b_qkv.reshape(NH, 3, HD)

    in_maps = []
    all_kLoT = None
    for c in range(NCORES):
        heads = [c + NCORES * j for j in range(HPC)]
        hs = np.asarray(heads)
        Wq = wr[hs, 0].reshape(DPC, H) * INV_NORM
        Wk = wr[hs, 1].reshape(DPC, H)
        Wv = wr[hs, 2].reshape(DPC, H)
        WQK = np.concatenate([Wq, Wk], axis=0)           # [1024, H]
        wqk_t = np.ascontiguousarray(
            WQK.reshape(2 * HPC, P, KT, P).transpose(0, 3, 2, 1)
        ).reshape(2 * HPC, P, KT * P)
        wqk_chi, wqk_clo = _split8(wqk_t, SW)
        wv_t = np.ascontiguousarray(
            Wv.reshape(DPC, KT, P).transpose(2, 1, 0)
        ).reshape(P, KT * DPC)
        wv_chi, wv_clo = _split8(wv_t, SW)
        bq = br[hs, 0].reshape(-1) * INV_NORM
        bk = br[hs, 1].reshape(-1)
        bqk_c = np.ascontiguousarray(
            np.concatenate([bq, bk]).reshape(2 * HPC, P).T)
        bv_c = br[hs, 2].reshape(1, DPC)

        al_c = alibi[hs].astype(np.float32)               # [HPC, S]
        # exact 3-way bf16 split of alibi
        a_hi = al_c.astype(bf).astype(np.float32)
        r1 = al_c - a_hi
        a_mid = r1.astype(bf).astype(np.float32)
        a_lo = (r1 - a_mid).astype(bf)
        alsp_c = np.stack(
            [a_hi.astype(bf), a_mid.astype(bf), a_lo], axis=1)  # [HPC,3,S]

        cmax = np.maximum.accumulate(al_c, axis=1)
        bexp_c = np.zeros((P, HPC * QB), np.float32)
        kLoT_c = []
        for h in range(HPC):
            b_row = cmax[h, limit - 1] + CPAD
            klo_h = []
            for qb in range(QB):
                bexp_c[:, h * QB + qb] = -b_row[qb * P:(qb + 1) * P]
                # keys whose softmax weight is < ~e^-19 for every query in
                # the block (qk slack 25 + prob floor e^-23): contribute
                # < 1e-5 total mass, far below the kernel's 5e-3 error
                bmin = b_row[qb * P:(qb + 1) * P].min()
                live = al_c[h] >= (bmin - 40.0)
                k0 = int(np.argmax(live)) if live.any() else 0
                klo_h.append(min(k0 // P, kNq[qb] // P - 1))
            kLoT_c.append(tuple(klo_h))
        kLoT_c = tuple(kLoT_c)
        if all_kLoT is None:
            all_kLoT = kLoT_c
        else:
            # one SPMD program for all cores: take the elementwise min
            all_kLoT = tuple(
                tuple(min(a, b) for a, b in zip(ra, rb))
                for ra, rb in zip(all_kLoT, kLoT_c))
        dcols = np.concatenate(
            [np.arange(g * HD, (g + 1) * HD) for g in heads])
        wdp_t = np.ascontiguousarray(
            w_dense[:, dcols].reshape(H, HPC, P)
            .transpose(2, 1, 0)).reshape(P, HPC * H)
        wdp_chi, wdp_clo = _split8(wdp_t, SW)
        in_maps.append({
            "hpk_hi": hpk_hi, "hpk_lo": hpk_lo,
            "wqk_hi": wqk_chi, "wqk_lo": wqk_clo,
            "wv_hi": wv_chi, "wv_lo": wv_clo,
            "bqk": bqk_c, "bv": bv_c,
            "alsp": alsp_c, "trit": trit, "bexp": bexp_c, "ident": ident,
            "wdp_hi": wdp_chi, "wdp_lo": wdp_clo,
        })
    return (tuple(kNq), all_kLoT), in_maps


def kernel(hidden_states, residual, alibi, attention_mask,
           w_qkv, b_qkv, w_dense, b_dense):
    key, in_maps = _host_prep(hidden_states, alibi, attention_mask,
                              w_qkv, b_qkv, w_dense)
    if key not in _CACHE:
        _CACHE[key] = _build(key)
    nc = _CACHE[key]
    res = run_bass_kernel_spmd(nc, in_maps, list(range(NCORES)))
    acc = res.results[0]["out_part"].astype(np.float32)
    for c in range(1, NCORES):
        acc += res.results[c]["out_part"].astype(np.float32)
    out = acc + np.asarray(b_dense, np.float32)[None, :]
    out = out + np.asarray(residual, np.float32).reshape(S, H)
    return out.reshape(B, S, H).astype(np.float32)


# revision 23
# speedup vs baseline: 1.2352x; 1.0272x over previous
"""BloomAttention (B=1, S=2048, H=4096, NH=32) on 8 Trainium2 cores — v4.

Head-parallel TP (4 heads/core). v4 converts the two big dense GEMMs (QKV
projection and output dense) to fp8-e4m3 DoubleRow matmuls with an exact
hi+lo error split (3-term: hi*hi + hi*lo + lo*hi), halving PE time per
term pair vs bf16 for a net 0.75x on those stages:
 - QKV: hidden (x8) and weights (x512) split to fp8 hi/lo on the host;
   48 DoubleRow matmuls (16 k-pairs x 3 terms) accumulate the full 4096
   contraction in PSUM; ACT descales by 1/4096 while adding the bias.
 - V: same; descale+bias in one DVE scalar_tensor_tensor op.
 - scores: bf16 QK matmul + 3-row aux matmul (ones3 x alibi split) +
   tri^T x identity for the causal diagonal block, all in PSUM.
 - softmax: exp from PSUM on ACT with host-precomputed per-query upper
   bound as bias; accum_out row sums; DVE normalize; PE transposes.
 - ctx: PV in bf16; PSUM result split on-device to fp8 hi/lo (copy +
   subtract) for the dense stage.
 - dense: fp8x3 DoubleRow over hh-pairs, row-parallel partials written
   bf16 (1/512 descale on the PSUM->stage copies); host sums cores+bias+
   residual.
"""
import math
import numpy as np
from contextlib import ExitStack

import ml_dtypes

import concourse.bacc as bacc
import concourse.bass as bass
import concourse.mybir as mybir
import concourse.tile as tile
from concourse.bass_utils import run_bass_kernel_spmd

B, S, H, NH = 1, 2048, 4096, 32
HD = H // NH            # 128
NCORES = 8
HPC = NH // NCORES      # 4 heads per core
DPC = HPC * HD          # 512
INV_NORM = 1.0 / math.sqrt(HD)
NEG = -1.0e30
CPAD = 15.0             # slack above max alibi in b_q
P = 128
QB = S // P             # 16 query blocks
NCH = 8                 # seq chunks in phase 1
SCW = S // NCH          # 256 seq chunk width
KT = H // P             # 32 contraction tiles
KP = KT // 2            # 16 DoubleRow k-pairs
SX = 8.0                # fp8 scale for hidden
SW = 512.0              # fp8 scale for weights
DSC = 1.0 / (SX * SW)   # PSUM descale
# per-term scales for the 6-way fp8 alibi split (weights 1/s are e4m3-exact)
ASCALES = (0.125, 2.0, 32.0, 512.0, 512.0, 512.0)
F32 = mybir.dt.float32
BF16 = mybir.dt.bfloat16
FP8 = mybir.dt.float8e4
ADD = mybir.AluOpType.add
MUL = mybir.AluOpType.mult
DR = mybir.MatmulPerfMode.DoubleRow

USE_DMA_T = False        # DMA X-bar transposes regress: ~160 extra DMAs
                         # at 625ns HWDGE + 650ns DGE delay each gate the
                         # chain->PV pipeline; PE transposes win

_CACHE = {}


def _build(key):
    kNq, kLoT = key
    nc = bacc.Bacc("TRN2", target_bir_lowering=False, debug=False,
                   num_devices=NCORES)

    hpk_hi = nc.dram_tensor("hpk_hi", [NCH, P, KT * SCW], FP8,
                            kind="ExternalInput")
    hpk_lo = nc.dram_tensor("hpk_lo", [NCH, P, KT * SCW], FP8,
                            kind="ExternalInput")
    wqk_hi = nc.dram_tensor("wqk_hi", [2 * HPC, P, KT * P], FP8,
                            kind="ExternalInput")
    wqk_lo = nc.dram_tensor("wqk_lo", [2 * HPC, P, KT * P], FP8,
                            kind="ExternalInput")
    wv_hi = nc.dram_tensor("wv_hi", [P, KT * DPC], FP8, kind="ExternalInput")
    wv_lo = nc.dram_tensor("wv_lo", [P, KT * DPC], FP8, kind="ExternalInput")
    bqk_t = nc.dram_tensor("bqk", [P, 2 * HPC], F32, kind="ExternalInput")
    bv_t = nc.dram_tensor("bv", [1, DPC], F32, kind="ExternalInput")
    alsp_t = nc.dram_tensor("alsp", [HPC, 3, 2, S], FP8, kind="ExternalInput")
    aw_t = nc.dram_tensor("aw", [P, 2 * P], FP8, kind="ExternalInput")
    trit_t = nc.dram_tensor("trit", [QB, P, P], BF16, kind="ExternalInput")
    bexp_t = nc.dram_tensor("bexp", [P, HPC * QB], F32, kind="ExternalInput")
    ident_t = nc.dram_tensor("ident", [P, P], BF16, kind="ExternalInput")
    wdp_hi = nc.dram_tensor("wdp_hi", [P, HPC * H], FP8, kind="ExternalInput")
    wdp_lo = nc.dram_tensor("wdp_lo", [P, HPC * H], FP8, kind="ExternalInput")
    out_t = nc.dram_tensor("out_part", [S, H], BF16, kind="ExternalOutput")

    Ident = mybir.ActivationFunctionType.Identity
    Exp = mybir.ActivationFunctionType.Exp

    with tile.TileContext(nc) as tc, ExitStack() as top:
        persist = top.enter_context(tc.tile_pool(name="persist", bufs=1))
        qk_sb = [persist.tile([P, S], BF16, tag=f"qk_{f}", name=f"qk_{f}")
                 for f in range(2 * HPC)]                  # Q heads 0-3, K heads 0-3
        v_sb = [persist.tile([P, DPC], BF16, tag=f"v_{st}", name=f"v_{st}")
                for st in range(S // P)]
        ident_sb = persist.tile([P, P], BF16, tag="ident")
        bqk_sb = persist.tile([P, 2 * HPC], F32, tag="bqk")
        bexp_sb = persist.tile([P, HPC * QB], F32, tag="bexp")
        bv_bc = persist.tile([P, DPC], F32, tag="bv_bc")
        aw_sb = persist.tile([P, 2, P], FP8, tag="aw")
        alsp_a = persist.tile([P, 2, S], FP8, tag="alsp_a")
        alsp_b = persist.tile([3, 2, S], FP8, tag="alsp_b")
        aw6 = [aw_sb[32 * h:32 * h + 3, :, :] for h in range(3)] + \
            [aw_sb[0:3, :, :]]
        alsp_sb = [alsp_a[32 * h:32 * h + 3, :, :] for h in range(3)] + \
            [alsp_b]

        trit_sb = [persist.tile([P, P], BF16, tag=f"trit_{qb}",
                                name=f"trit_{qb}") for qb in range(QB)]
        # ctx in fp8 hi/lo, hh-major so hh-pairs are stride-S slices
        ctx_hi = persist.tile([P, HPC, S], FP8, tag="ctx_hi")
        ctx_lo = persist.tile([P, HPC, S], FP8, tag="ctx_lo")
        sm_pool = top.enter_context(tc.tile_pool(name="small", bufs=8))

        prow_t = {}   # (h, qb) -> prob row tile
        quads = {}    # (h, qg) -> dict of quad tiles [P, 4, 512]

        def chain(h, qb, pool, mid_cb=None):
            """scores (+alibi, -b_q, mask) in PSUM -> exp -> normalized
            bf16 prob row. mid_cb (if given) is invoked after the second
            chunk so PE has filler work while exp drains the PSUM tiles."""
            kN = kNq[qb]
            lo0 = (kLoT[h][qb] * P) // 512 * 512
            nt = (kN - lo0 + 511) // 512
            prow = pool.tile([P, 512 if qb < 4 else S], BF16, tag="prow",
                             name=f"prow_{h}_{qb}")
            prow_t[(h, qb)] = prow
            strip = sm_pool.tile([P, 4], F32, tag="strip")
            rinv = sm_pool.tile([P, 1], F32, tag="rinv")
            nb = bexp_sb[:, h * QB + qb: h * QB + qb + 1]
            qst = qk_sb[h][:, qb * P:(qb + 1) * P]
            for ti in range(nt):
                lo = lo0 + 512 * ti
                N = min(512, kN - lo)
                ps = ps_sc.tile([P, 512], F32, tag="ps_sc")
                sl = ps[:, :N]
                diag = (lo + N == kN)
                nc.tensor.matmul(
                    sl, qst, qk_sb[HPC + h][:, lo:lo + N],
                    start=True, stop=False)
                # alibi: one fp8 DoubleRow per 256 cols (6-term exact split,
                # per-slot power-of-2 stationary weights)
                for n0 in range(0, N, 256):
                    n1 = min(n0 + 256, N)
                    nc.tensor.matmul(
                        sl[:, n0:n1], aw6[h],
                        alsp_sb[h][:, :, lo + n0:lo + n1],
                        start=False,
                        stop=(not diag) and (n1 == N),
                        perf_mode=DR)
                if diag:
                    nc.tensor.matmul(
                        ps[:, N - P:N],
                        trit_sb[qb], ident_sb,
                        start=False, stop=True)
                nc.scalar.activation(
                    out=prow[:, lo:lo + N], in_=sl,
                    func=Exp, bias=nb, scale=1.0,
                    accum_out=strip[:, ti:ti + 1])
                if mid_cb is not None and ti == min(1, nt - 1):
                    mid_cb()
                    mid_cb = None
            if mid_cb is not None:
                mid_cb()
            if nt > 1:
                tot = sm_pool.tile([P, 1], F32, tag="tot")
                nc.vector.tensor_reduce(
                    out=tot, in_=strip[:, :nt], op=ADD,
                    axis=mybir.AxisListType.X)
            else:
                tot = strip[:, 0:1]
            nc.vector.reciprocal(out=rinv, in_=tot)
            nc.vector.tensor_scalar_mul(
                out=prow[:, lo0:kN], in0=prow[:, lo0:kN], scalar1=rinv)

        # ---------------- phase 1: QKV projection ----------------
        with ExitStack() as ph1:
            wq_pool = ph1.enter_context(tc.tile_pool(name="wq", bufs=1))
            hid_pool = ph1.enter_context(tc.tile_pool(name="hid", bufs=2))
            psqk = ph1.enter_context(
                tc.tile_pool(name="psqk", bufs=5, space="PSUM"))
            psv = ph1.enter_context(
                tc.tile_pool(name="psv", bufs=3, space="PSUM"))

            wqkh_sb = [wq_pool.tile([P, KT, P], FP8, tag=f"wqkh_{f}",
                                    name=f"wqkh_{f}") for f in range(2 * HPC)]
            wqkl_sb = [wq_pool.tile([P, KT, P], FP8, tag=f"wqkl_{f}",
                                    name=f"wqkl_{f}") for f in range(2 * HPC)]
            wvh_sb = wq_pool.tile([P, KT, DPC], FP8, tag="wvh")
            wvl_sb = wq_pool.tile([P, KT, DPC], FP8, tag="wvl")
            hph = [hid_pool.tile([P, KT, SCW], FP8, tag="hph",
                                 name=f"hph_{c}") for c in range(NCH)]
            hpl = [hid_pool.tile([P, KT, SCW], FP8, tag="hpl",
                                 name=f"hpl_{c}") for c in range(NCH)]
            # chunk-0's operands stream in use-order: first k-pair of hi
            # weights + hidden (hi*hi jp0 starts ~0.5us in), bias, then the
            # rest hi, then lo tensors
            q8 = 8 * SCW
            w16 = 16 * P
            nc.sync.dma_start(out=wqkh_sb[0][:, :2, :],
                              in_=wqk_hi[0][:, :2 * P])
            nc.sync.dma_start(out=hph[0][:, :2, :],
                              in_=hpk_hi[0][:, :2 * SCW])
            nc.sync.dma_start(out=bqk_sb, in_=bqk_t[:, :])
            nc.sync.dma_start(out=wqkh_sb[0][:, 2:16, :],
                              in_=wqk_hi[0][:, 2 * P:w16])
            nc.sync.dma_start(out=hph[0][:, 2:8, :],
                              in_=hpk_hi[0][:, 2 * SCW:q8])
            nc.sync.dma_start(out=wqkh_sb[0][:, 16:, :],
                              in_=wqk_hi[0][:, w16:])
            nc.sync.dma_start(out=hph[0][:, 8:16, :],
                              in_=hpk_hi[0][:, q8:2 * q8])
            nc.sync.dma_start(out=hph[0][:, 16:, :], in_=hpk_hi[0][:, 2 * q8:])
            nc.sync.dma_start(out=wqkl_sb[0], in_=wqk_lo[0])
            nc.sync.dma_start(out=hpl[0][:, :16, :], in_=hpk_lo[0][:, :2 * q8])
            nc.sync.dma_start(out=hpl[0][:, 16:, :], in_=hpk_lo[0][:, 2 * q8:])
            # each fc group consumes hi+lo weights in ~2.5us; keep the queue
            # in exactly that order so chunk-0 groups never stall long
            for f in range(1, 2 * HPC):
                nc.sync.dma_start(out=wqkh_sb[f], in_=wqk_hi[f])
                nc.sync.dma_start(out=wqkl_sb[f], in_=wqk_lo[f])
            # chunk-1 hidden BEFORE wv: chunk-1 QK is the next PE consumer;
            # V(0) runs after chunk-1 QK so wv can trail it
            nc.sync.dma_start(out=hph[1], in_=hpk_hi[1])
            nc.sync.dma_start(out=hpl[1], in_=hpk_lo[1])
            half = KT * DPC // 2
            nc.sync.dma_start(out=wvh_sb[:, :16, :], in_=wv_hi[:, :half])
            nc.sync.dma_start(out=wvh_sb[:, 16:, :], in_=wv_hi[:, half:])
            nc.sync.dma_start(out=wvl_sb[:, :16, :], in_=wv_lo[:, :half])
            nc.sync.dma_start(out=wvl_sb[:, 16:, :], in_=wv_lo[:, half:])
            nc.gpsimd.dma_start(
                out=bv_bc,
                in_=bass.AP(tensor=bv_t, offset=0, ap=[[0, P], [1, DPC]]))

            def v_groups(c):
                xh, xl = hph[c], hpl[c]
                for st2 in range(SCW // P):
                    st = c * (SCW // P) + st2
                    ps = psv.tile([P, DPC], F32, tag="psv")
                    for vh in range(2):
                        psl = ps[:, vh * 256:(vh + 1) * 256]
                        terms = [(xh, wvh_sb), (xh, wvl_sb), (xl, wvh_sb)]
                        for i3, (x3, w3) in enumerate(
                                (x, w) for (x, w) in terms
                                for _ in range(KP)):
                            jp = i3 % KP
                            nc.tensor.matmul(
                                psl,
                                x3[:, 2 * jp:2 * jp + 2,
                                   st2 * P:(st2 + 1) * P],
                                w3[:, 2 * jp:2 * jp + 2,
                                   vh * 256:(vh + 1) * 256],
                                start=(i3 == 0), stop=(i3 == 3 * KP - 1),
                                perf_mode=DR)
                    nc.vector.scalar_tensor_tensor(
                        out=v_sb[st], in0=ps, scalar=DSC, in1=bv_bc,
                        op0=MUL, op1=ADD)

            for c in range(NCH):
                xh, xl = hph[c], hpl[c]
                if 0 < c < NCH - 1:
                    nc.sync.dma_start(out=hph[c + 1], in_=hpk_hi[c + 1])
                    nc.sync.dma_start(out=hpl[c + 1], in_=hpk_lo[c + 1])
                if c == 1:
                    # attention-phase constants; emitted here so they queue
                    # behind the first hidden chunks, not ahead of them
                    nc.sync.dma_start(out=ident_sb, in_=ident_t[:, :])
                    nc.sync.dma_start(out=bexp_sb, in_=bexp_t[:, :])
                    nc.sync.dma_start(out=aw_sb, in_=aw_t[:, :])
                    for qb in range(QB):
                        nc.sync.dma_start(out=trit_sb[qb], in_=trit_t[qb])
                    for h in range(HPC):
                        nc.sync.dma_start(out=alsp_sb[h], in_=alsp_t[h])
                for f in range(2 * HPC):
                    ps = psqk.tile([P, SCW], F32, tag="psqk")
                    terms = [(wqkh_sb[f], xh), (wqkl_sb[f], xh),
                             (wqkh_sb[f], xl)]
                    n3 = 3 * KP
                    for i3, (w3, x3) in enumerate(
                            (w, x) for (w, x) in terms for _ in range(KP)):
                        jp = i3 % KP
                        nc.tensor.matmul(
                            ps, w3[:, 2 * jp:2 * jp + 2, :],
                            x3[:, 2 * jp:2 * jp + 2, :],
                            start=(i3 == 0), stop=(i3 == n3 - 1),
                            perf_mode=DR)
                    nc.scalar.activation(
                        out=qk_sb[f][:, c * SCW:(c + 1) * SCW], in_=ps,
                        func=Ident, bias=bqk_sb[:, f:f + 1], scale=DSC)
                # chunk c-1's V groups run here: their wv / hp-lo operands
                # have had a full extra chunk of DMA time to arrive, and the
                # hp buffers (bufs=2) are still alive
                if c > 0:
                    v_groups(c - 1)
            v_groups(NCH - 1)

        # ---------------- phases 2+3: attention + dense ----------------
        with ExitStack() as ph2:
            pr_pool = ph2.enter_context(tc.tile_pool(name="prow", bufs=9))
            pq_pool = ph2.enter_context(tc.tile_pool(name="pquad", bufs=9))
            wd_pool = ph2.enter_context(tc.tile_pool(name="wd", bufs=1))
            st_pool = ph2.enter_context(tc.tile_pool(name="ostage", bufs=2))
            ps_sc = ph2.enter_context(
                tc.tile_pool(name="ps_sc", bufs=3, space="PSUM"))
            ps_cx = ph2.enter_context(
                tc.tile_pool(name="ps_cx", bufs=1, space="PSUM"))
            ps_d = ph2.enter_context(
                tc.tile_pool(name="ps_d", bufs=2, space="PSUM"))
            if not USE_DMA_T:
                ps_st = ph2.enter_context(
                    tc.tile_pool(name="ps_st", bufs=2, space="PSUM"))

            wdh_sb = wd_pool.tile([P, HPC, H], FP8, tag="wdh")
            wdl_sb = wd_pool.tile([P, HPC, H], FP8, tag="wdl")
            nc.sync.dma_start(out=wdh_sb, in_=wdp_hi[:, :])
            nc.sync.dma_start(out=wdl_sb, in_=wdp_lo[:, :])


            def transposes(h, qb):
                """prow(h, qb) -> key-major quad slices."""
                kN = kNq[qb]
                qg, qbl = qb // 4, qb % 4
                if qbl == 0:
                    ntile_g = kNq[4 * qg + 3] // P
                    a0 = kLoT[h][4 * qg] // 4
                    quads[(h, qg)] = {
                        a: pq_pool.tile([P, 4, 512], BF16, tag="pquad",
                                        name=f"pq_{h}_{qg}_{a}")
                        for a in range(a0, (ntile_g + 3) // 4)}
                prow = prow_t.pop((h, qb))
                qlist = quads[(h, qg)]
                ntile = kN // P
                t0 = kLoT[h][qb]
                if USE_DMA_T:
                    # batched X-bar transpose: one DMA per quad covers up to
                    # 4 key tiles; dst[p, t, c] = prow[c, t*P + p]
                    t = t0
                    while t < ntile:
                        a = t // 4
                        hi_t = min(4 * a + 4, ntile)
                        nc.sync.dma_start(
                            out=qlist[a][:, t - 4 * a:hi_t - 4 * a,
                                         qbl * P:(qbl + 1) * P],
                            in_=prow[:, t * P:hi_t * P], transpose=True)
                        t = hi_t
                else:
                    t = t0
                    while t < ntile:
                        t = (t // 4) * 4          # align to quad boundary
                        lo_t = max(t, t0)
                        cnt = min(8, ntile - t)
                        stg = ps_st.tile([P, 8, P], BF16, tag="stg")
                        for i in range(lo_t - t, cnt):
                            nc.tensor.transpose(
                                stg[:, i, :], prow[:, (t + i) * P:(t + i + 1) * P],
                                ident_sb)
                        for half in range((cnt + 3) // 4):
                            i0 = max(4 * half, lo_t - t)
                            i1 = min(4 * half + 4, cnt)
                            if i0 >= i1:
                                continue
                            nc.vector.tensor_copy(
                                out=qlist[t // 4 + half][:, i0 - 4 * half:
                                                         i1 - 4 * half,
                                                         qbl * P:(qbl + 1) * P],
                                in_=stg[:, i0:i1, :])
                        t += cnt

            def pv(h, qg):
                kns = [kNq[4 * qg + i] for i in range(4)]
                t0s = [kLoT[h][4 * qg + i] for i in range(4)]
                ntile = kns[3] // P
                tiles_per = [k // P for k in kns]
                qlist = quads.pop((h, qg))
                cps = ps_cx.tile([P, 512], F32, tag="ps_cx")
                nc.vector.memset(cps, 0.0)
                for t in range(t0s[0], ntile):
                    cov = [i for i in range(4)
                           if tiles_per[i] > t and t0s[i] <= t]
                    if not cov:
                        continue
                    ilo, ihi = cov[0], cov[-1]
                    nc.tensor.matmul(
                        cps[:, ilo * P:(ihi + 1) * P],
                        v_sb[t][:, h * P:(h + 1) * P],
                        qlist[t // 4][:, t % 4, ilo * P:(ihi + 1) * P],
                        start=False, stop=(t == ntile - 1),
                        skip_group_check=True)
                nc.vector.tensor_copy(
                    out=ctx_hi[:, h, qg * 512:(qg + 1) * 512], in_=cps)
                nc.vector.tensor_sub(
                    out=ctx_lo[:, h, qg * 512:(qg + 1) * 512], in0=cps,
                    in1=ctx_hi[:, h, qg * 512:(qg + 1) * 512])

            def dense_qb(qb, split_dma=False):
                    stage = st_pool.tile([P, H], BF16, tag="ostage")
                    for oc in range(8):
                        ps = ps_d.tile([P, 512], F32, tag="ps_d")
                        for dh in range(2):
                            psl = ps[:, dh * 256:(dh + 1) * 256]
                            o0 = oc * 512 + dh * 256
                            i3 = 0
                            for hp2 in (0, 2):
                                for (ca, wb) in ((ctx_hi, wdh_sb),
                                                 (ctx_hi, wdl_sb),
                                                 (ctx_lo, wdh_sb)):
                                    nc.tensor.matmul(
                                        psl,
                                        ca[:, hp2:hp2 + 2,
                                           qb * P:(qb + 1) * P],
                                        wb[:, hp2:hp2 + 2, o0:o0 + 256],
                                        start=(i3 == 0), stop=(i3 == 5),
                                        perf_mode=DR)
                                    i3 += 1
                        if oc % 2 == 0:
                            nc.vector.tensor_scalar_mul(
                                out=stage[:, oc * 512:(oc + 1) * 512],
                                in0=ps, scalar1=1.0 / SW)
                        else:
                            nc.scalar.mul(
                                out=stage[:, oc * 512:(oc + 1) * 512],
                                in_=ps, mul=1.0 / SW)
                        if split_dma and oc == 3:
                            nc.sync.dma_start(
                                out=out_t[qb * P:(qb + 1) * P, :H // 2],
                                in_=stage[:, :H // 2])
                    if split_dma:
                        nc.sync.dma_start(
                            out=out_t[qb * P:(qb + 1) * P, H // 2:],
                            in_=stage[:, H // 2:])
                    else:
                        nc.sync.dma_start(
                            out=out_t[qb * P:(qb + 1) * P, :], in_=stage)

            # software-pipelined emission over 16 (qg, h) units, head-major
            # within each query group. Unit-level lag: unit u's four chains
            # are interleaved with unit u-1's transposes; u-1's PV closes at
            # qbl 3. Dense query blocks are spread one-or-two per unit as
            # their query group's ctx completes.
            units = [(qg, h) for qg in range(4) for h in range(HPC)]
            dq = []        # dense qbs ready to emit
            for u, (qg, h) in enumerate(units):
                prev = units[u - 1] if u > 0 else None
                for qbl in range(4):
                    steps = []
                    if prev:
                        steps.append(lambda ph=prev[1], pq=4 * prev[0] + qbl:
                                     transposes(ph, pq))
                    if qbl == 2 and dq:
                        steps.append(lambda q=dq.pop(0): dense_qb(q))
                    cb = (lambda ss=tuple(steps): [s() for s in ss]) \
                        if steps else None
                    chain(h, 4 * qg + qbl, pr_pool, mid_cb=cb)
                    if qbl == 3 and prev:
                        pv(prev[1], prev[0])
                        if prev[1] == HPC - 1:
                            dq.extend(4 * prev[0] + i for i in range(4))
                if len(dq) > 4:
                    dense_qb(dq.pop(0))
            prev = units[-1]
            for qbl in range(4):
                transposes(prev[1], 4 * prev[0] + qbl)
            pv(prev[1], prev[0])
            dq.extend(4 * prev[0] + i for i in range(4))
            for i, qb in enumerate(dq):
                dense_qb(qb, split_dma=(i == len(dq) - 1))

    nc.compile()
    return nc


def _split8(x, sc):
    """Exact hi+lo fp8-e4m3 split of x*sc. Returns (hi, lo) fp8 arrays."""
    f8 = ml_dtypes.float8_e4m3
    hi = (x * sc).astype(f8)
    lo = (x * sc - hi.astype(np.float32)).astype(f8)
    return hi, lo


def _host_prep(hidden_states, alibi, attention_mask, w_qkv, b_qkv, w_dense):
    """Returns (kNq, in_maps) for the 8 cores."""
    hidden = np.asarray(hidden_states, np.float32).reshape(S, H)
    mask = np.asarray(attention_mask).reshape(S, S)
    alibi = np.asarray(alibi, np.float32).reshape(NH, S)
    w_qkv = np.asarray(w_qkv, np.float32)
    b_qkv = np.asarray(b_qkv, np.float32)
    w_dense = np.asarray(w_dense, np.float32)

    allowed = ~mask
    assert allowed.any(axis=1).all(), "fully-masked row"
    limit = S - np.argmax(allowed[:, ::-1], axis=1)      # last allowed + 1
    recon = np.arange(S)[None, :] >= limit[:, None]
    if not np.array_equal(mask, recon):
        raise NotImplementedError("mask is not suffix-structured")
    kNq = []
    for qb in range(QB):
        lb = limit[qb * P:(qb + 1) * P]
        kN = int(math.ceil(lb.max() / P) * P)
        if lb.min() < kN - P:
            raise NotImplementedError("mask boundary spans >128 cols in block")
        kNq.append(kN)
    if any(kNq[i] > kNq[i + 1] for i in range(QB - 1)):
        raise NotImplementedError("non-monotone key ranges")

    bf = ml_dtypes.bfloat16
    hp_t = np.ascontiguousarray(
        hidden.reshape(NCH, SCW, KT, P).transpose(0, 3, 2, 1)
    ).reshape(NCH, P, KT * SCW)
    hpk_hi, hpk_lo = _split8(hp_t, SX)
    ident = np.eye(P, dtype=np.float32).astype(bf)
    col = np.arange(S)

    # alibi-sum stationary weights: rows (32h+r) x slots i = 1/ASCALES[3i+r]
    aw = np.zeros((P, 2, P), np.float32)
    for hoff in range(0, P, 32):
        for r in range(3):
            aw[hoff + r, 0, :] = 1.0 / ASCALES[r]
            aw[hoff + r, 1, :] = 1.0 / ASCALES[3 + r]
    aw = aw.reshape(P, 2 * P).astype(ml_dtypes.float8_e4m3)

    # causal diagonal mask tiles, transposed for use as matmul stationary:
    # trit[qb][k, q] = NEG where key kN-P+k is masked for query q
    trit = np.zeros((QB, P, P), np.float32)
    for qb in range(QB):
        kN = kNq[qb]
        lb = limit[qb * P:(qb + 1) * P]
        cc = col[kN - P:kN]
        trit[qb] = np.where(cc[:, None] >= lb[None, :], NEG, 0.0)
    trit = trit.astype(bf)

    wr = w_qkv.reshape(NH, 3, HD, H)
    br = b_qkv.reshape(NH, 3, HD)

    in_maps = []
    all_kLoT = None
    for c in range(NCORES):
        heads = [c + NCORES * j for j in range(HPC)]
        hs = np.asarray(heads)
        Wq = wr[hs, 0].reshape(DPC, H) * INV_NORM
        Wk = wr[hs, 1].reshape(DPC, H)
        Wv = wr[hs, 2].reshape(DPC, H)
        WQK = np.concatenate([Wq, Wk], axis=0)           # [1024, H]
        wqk_t = np.ascontiguousarray(
            WQK.reshape(2 * HPC, P, KT, P).transpose(0, 3, 2, 1)
        ).reshape(2 * HPC, P, KT * P)
        wqk_chi, wqk_clo = _split8(wqk_t, SW)
        wv_t = np.ascontiguousarray(
            Wv.reshape(DPC, KT, P).transpose(2, 1, 0)
        ).reshape(P, KT * DPC)
        wv_chi, wv_clo = _split8(wv_t, SW)
        bq = br[hs, 0].reshape(-1) * INV_NORM
        bk = br[hs, 1].reshape(-1)
        bqk_c = np.ascontiguousarray(
            np.concatenate([bq, bk]).reshape(2 * HPC, P).T)
        bv_c = br[hs, 2].reshape(1, DPC)

        al_c = alibi[hs].astype(np.float32)               # [HPC, S]
        # 6-term fp8 split of alibi: term j at scale ASCALES[j], summed on
        # the PE with stationary weight 1/ASCALES[j] (all e4m3-exact)
        f8 = ml_dtypes.float8_e4m3
        alsp_c = np.zeros((HPC, 3, 2, S), f8)
        rres = al_c.copy()
        for j, sc in enumerate(ASCALES):
            q = (rres * sc).astype(f8)
            alsp_c[:, j % 3, j // 3, :] = q
            rres -= q.astype(np.float32) / sc

        cmax = np.maximum.accumulate(al_c, axis=1)
        bexp_c = np.zeros((P, HPC * QB), np.float32)
        kLoT_c = []
        for h in range(HPC):
            b_row = cmax[h, limit - 1] + CPAD
            klo_h = []
            for qb in range(QB):
                bexp_c[:, h * QB + qb] = -b_row[qb * P:(qb + 1) * P]
                # keys whose softmax weight is < ~e^-19 for every query in
                # the block (qk slack 25 + prob floor e^-23): contribute
                # < 1e-5 total mass, far below the kernel's 5e-3 error
                bmin = b_row[qb * P:(qb + 1) * P].min()
                live = al_c[h] >= (bmin - 40.0)
                k0 = int(np.argmax(live)) if live.any() else 0
                klo_h.append(min(k0 // P, kNq[qb] // P - 1))
            kLoT_c.append(tuple(klo_h))
        kLoT_c = tuple(kLoT_c)
        if all_kLoT is None:
            all_kLoT = kLoT_c
        else:
            # one SPMD program for all cores: take the elementwise min
            all_kLoT = tuple(
                tuple(min(a, b) for a, b in zip(ra, rb))
                for ra, rb in zip(all_kLoT, kLoT_c))
        dcols = np.concatenate(
            [np.arange(g * HD, (g + 1) * HD) for g in heads])
        wdp_t = np.ascontiguousarray(
            w_dense[:, dcols].reshape(H, HPC, P)
            .transpose(2, 1, 0)).reshape(P, HPC * H)
        wdp_chi, wdp_clo = _split8(wdp_t, SW)
        in_maps.append({
            "hpk_hi": hpk_hi, "hpk_lo": hpk_lo,
            "wqk_hi": wqk_chi, "wqk_lo": wqk_clo,
            "wv_hi": wv_chi, "wv_lo": wv_clo,
            "bqk": bqk_c, "bv": bv_c,
            "alsp": alsp_c, "aw": aw, "trit": trit, "bexp": bexp_c,
            "ident": ident,
            "wdp_hi": wdp_chi, "wdp_lo": wdp_clo,
        })
    return (tuple(kNq), all_kLoT), in_maps


def kernel(hidden_states, residual, alibi, attention_mask,
           w_qkv, b_qkv, w_dense, b_dense):
    key, in_maps = _host_prep(hidden_states, alibi, attention_mask,
                              w_qkv, b_qkv, w_dense)
    if key not in _CACHE:
        _CACHE[key] = _build(key)
    nc = _CACHE[key]
    res = run_bass_kernel_spmd(nc, in_maps, list(range(NCORES)))
    acc = res.results[0]["out_part"].astype(np.float32)
    for c in range(1, NCORES):
        acc += res.results[c]["out_part"].astype(np.float32)
    out = acc + np.asarray(b_dense, np.float32)[None, :]
    out = out + np.asarray(residual, np.float32).reshape(S, H)
    return out.reshape(B, S, H).astype(np.float32)
